# revision 49
# baseline (speedup 1.0000x reference)
"""Multi-head attention + residual + LayerNorm on 8 Trainium2 NeuronCores.

Problem: B=4, S=2048, D=1024, 16 heads (hd=64), fp32 I/O.

Sharding (no collectives): 8 cores = 4 batches x 2 query-halves.
Core c handles batch b=c//2, query rows h*1024:(h+1)*1024 (h=c%2), producing
the full (1024, 1024) output slice for those tokens. K/V projections for the
whole 2048-token sequence of batch b are computed on both cores of the pair
(the only redundant compute, ~20%).

Per-core kernel layout (all matmul operands bf16, fp32 PSUM accumulate):
  - Q/K projections produce TRANSPOSED outputs QT/KT [d_out-part, token-free]
    so attention scores S^T = K_h @ Q_h^T need no on-chip transposes.
  - V projection produces normal layout V [token-part, d-free].
  - scores^T [k-tok, q-tok] per head -> exp (no max subtraction: |s/8| <~ 2
    for randn inputs) -> E^T bf16.
  - sum_k exp: matmul with a ones[128,1] stationary vector, 4 accumulation
    strips packed at PSUM partitions 0/32/64/96 via tile_position col packing.
  - context^T[d, q] = V_chunk.T-free matmuls accumulated over k chunks, two
    heads packed per PE pass at array columns 0-63 / 64-127.
  - out-projection back to normal layout, + residual + bias, LayerNorm.
  - bv is algebraically folded into the output bias on the host:
    bo2 = bo + bv @ Wo.T (softmax-normalized V bias contributes bv exactly).
"""

import sys

for _p in ("/opt/trn_rl_repo",):
    if _p not in sys.path:
        sys.path.insert(0, _p)

from contextlib import ExitStack

import numpy as np
import ml_dtypes

import concourse.bass as bass
import concourse.mybir as mybir
from concourse import bacc
from concourse.tile import TileContext

EMBED = 1024
HEADS = 16
HEAD_DIM = 64
B_FULL, S_FULL = 4, 2048
N_CORES = 8

f32 = mybir.dt.float32
bf16 = mybir.dt.bfloat16
FT = mybir.ActivationFunctionType


def build_mha(D, NH, SQ, SK, num_devices=N_CORES, dbg=False):
    """Build the per-core Bass module.

    D: embed dim, NH: heads, SQ: query tokens this core owns,
    SK: key/value tokens (full sequence of this core's batch).
    """
    HD = 64
    assert D % 128 == 0 and NH * HD == D
    KC = D // 128          # contraction chunks of 128 input features
    HP = NH // 2           # head-pair chunks (= D//128 output chunks)
    NG = HP // 2           # groups of 2 head pairs (V-proj at 256-col grain)
    SKT = SK // 128        # k-token chunks
    SQT = SQ // 128        # q-token tiles for out-proj/LN
    QN = max(1, SQ // 512)  # 512-wide q tiles
    QW = SQ // QN
    KN = max(1, SK // 512)
    KW = SK // KN
    EN = max(1, D // 512)  # 512-wide out-feature tiles
    EW = D // EN
    NSUB = max(1, D // 512)  # bn_stats subgroups

    nc = bacc.Bacc(
        "TRN2", target_bir_lowering=False, debug=False, num_devices=num_devices
    )

    dp = nc.declare_dram_parameter
    xqT = dp("xqT", [D, SQ], bf16, isOutput=False)
    xres = dp("xres", [SQ, D], f32, isOutput=False)
    xkT = dp("xkT", [D, SK], bf16, isOutput=False)
    xvT = dp("xvT", [D, SK], bf16, isOutput=False)
    wqTr = dp("wqTr", [HP * D, 128], bf16, isOutput=False)   # Wq.T hp-col slices
    wkTr = dp("wkTr", [HP * D, 128], bf16, isOutput=False)
    wvTr = dp("wvTr", [NG * D, 256], bf16, isOutput=False)   # Wv.T group slices
    woTr = dp("woTr", [D, D], bf16, isOutput=False)          # Wo.T
    bq_d = dp("bq", [D], f32, isOutput=False)
    bk_d = dp("bk", [D], f32, isOutput=False)
    bo2_d = dp("bo2", [1, D], f32, isOutput=False)           # bo + bv @ Wo.T
    gam_d = dp("gam", [1, D], f32, isOutput=False)
    bet_d = dp("bet", [1, D], f32, isOutput=False)
    out_d = dp("out", [SQ, D], f32, isOutput=True)
    if dbg:
        dbg_qt = dp("dbg_qt", [D, SQ], bf16, isOutput=True)
        dbg_kt = dp("dbg_kt", [D, SK], bf16, isOutput=True)
        dbg_v = dp("dbg_v", [SK, D], bf16, isOutput=True)
        dbg_ct = dp("dbg_ct", [D, SQ], bf16, isOutput=True)
        dbg_rc = dp("dbg_rc", [NH // 2, 97, SQ // max(1, SQ // 512)], f32, isOutput=True)
        dbg_cx = dp("dbg_cx", [D, SQ], f32, isOutput=True)
        dbg_rcb = dp("dbg_rcb", [NH // 2, 128, SQ], f32, isOutput=True)

    with TileContext(nc) as tc, ExitStack() as ctx:
        consts = ctx.enter_context(tc.tile_pool(name="consts", bufs=1))
        px = ctx.enter_context(tc.tile_pool(name="px", bufs=1))
        pw = ctx.enter_context(tc.tile_pool(name="pw", bufs=1))
        pqk = ctx.enter_context(tc.tile_pool(name="pqk", bufs=2))
        pv = ctx.enter_context(tc.tile_pool(name="pv", bufs=2))
        pct = ctx.enter_context(tc.tile_pool(name="pct", bufs=1))
        pe_ = ctx.enter_context(tc.tile_pool(name="pe", bufs=4))
        pmisc = ctx.enter_context(tc.tile_pool(name="pmisc", bufs=2))
        pio = ctx.enter_context(tc.tile_pool(name="pio", bufs=3))

        pdram = ctx.enter_context(tc.tile_pool(name="pdram", bufs=2, space="DRAM"))
        ppj = ctx.enter_context(tc.tile_pool(name="ppj", bufs=1, space="PSUM"))
        psc = ctx.enter_context(tc.tile_pool(name="psc", bufs=2, space="PSUM"))
        pcx = ctx.enter_context(tc.tile_pool(name="pcx", bufs=1, space="PSUM"))
        pse = ctx.enter_context(tc.tile_pool(name="pse", bufs=1, space="PSUM"))

        # ---- constants
        ones_bf = consts.tile([128, 1], bf16, tag="ones")
        nc.vector.memset(ones_bf, 1.0)
        eps_sb = consts.tile([128, 1], f32, tag="eps")
        nc.vector.memset(eps_sb, 1e-5)
        bq_sb = consts.tile([128, KC], f32, tag="bq")
        nc.sync.dma_start(out=bq_sb, in_=bq_d.rearrange("(c p) -> p c", p=128))
        bk_sb = consts.tile([128, KC], f32, tag="bk")
        nc.sync.dma_start(out=bk_sb, in_=bk_d.rearrange("(c p) -> p c", p=128))

        # ---- stage activations (bf16, pre-transposed on host)
        # xv first: the V projection is the first PE consumer, so its DMA
        # must win the queue race to shorten the kernel lead-in stall
        # chunk-split DMAs: first matmuls start after ~one chunk instead of
        # waiting for the whole monolithic transfer
        # xv split by TOKEN range (not feature chunk): each V-proj tok-tile
        # contracts over all KC feature chunks, so a token-range piece is the
        # unit that unblocks the first matmuls
        xv_sb = px.tile([128, KC, SK], bf16, tag="xv")
        for tr in range(4):
            ts_ = slice(tr * (SK // 4), (tr + 1) * (SK // 4))
            nc.sync.dma_start(
                out=xv_sb[:, :, ts_],
                in_=xvT[:, ts_].rearrange("(c p) m -> p c m", p=128),
            )

        def load_wv(g):
            t = pw.tile([128, KC, 256], bf16, tag="wv")
            nc.sync.dma_start(
                out=t,
                in_=wvTr[g * D:(g + 1) * D, :].rearrange("(c p) m -> p c m", p=128),
            )
            return t

        wv_t0 = load_wv(0)  # before xq/xk so the first PE consumer wins the queues

        xq_sb = px.tile([128, KC, SQ], bf16, tag="xq")
        for kc in range(KC):
            nc.sync.dma_start(
                out=xq_sb[:, kc, :], in_=xqT[kc * 128:(kc + 1) * 128, :]
            )
        xk_sb = px.tile([128, KC, SK], bf16, tag="xk")
        for kc in range(KC):
            nc.sync.dma_start(
                out=xk_sb[:, kc, :], in_=xkT[kc * 128:(kc + 1) * 128, :]
            )

        CT = pct.tile([128, HP, SQ], bf16, tag="ct")
        wo_sb = pct.tile([128, HP, D], bf16, tag="wo")

        # row constants replicated across all 128 partitions at DMA time
        # (stride-0 partition APs are DMA-only). Emitted after the hot input
        # transfers: 1.5MB of replicated writes only needed at the LN tail.
        bo2_sb = consts.tile([128, D], f32, tag="bo2")
        nc.sync.dma_start(out=bo2_sb, in_=bo2_d[:].to_broadcast((128, D)))
        gam_sb = consts.tile([128, D], f32, tag="gam")
        nc.sync.dma_start(out=gam_sb, in_=gam_d[:].to_broadcast((128, D)))
        bet_sb = consts.tile([128, D], f32, tag="bet")
        nc.sync.dma_start(out=bet_sb, in_=bet_d[:].to_broadcast((128, D)))

        def vproj_chunk(wv_t, vg, t):
            pj = ppj.tile([128, 256], f32, tag="pj")
            for kc in range(KC):
                nc.tensor.matmul(
                    pj,
                    lhsT=xv_sb[:, kc, t * 128:(t + 1) * 128],
                    rhs=wv_t[:, kc, :],
                    start=(kc == 0),
                    stop=(kc == KC - 1),
                )
            nc.any.tensor_copy(out=vg[:, t, :], in_=pj)

        def qproj_chunk(wq_t, qt_t, hp, qn):
            qs = slice(qn * QW, (qn + 1) * QW)
            pj = ppj.tile([128, QW], f32, tag="pj")
            for kc in range(KC):
                nc.tensor.matmul(
                    pj, lhsT=wq_t[:, kc, :], rhs=xq_sb[:, kc, qs],
                    start=(kc == 0), stop=(kc == KC - 1),
                )
            nc.any.tensor_tensor(
                out=qt_t[:, qs], in0=pj,
                in1=bq_sb[:, hp:hp + 1].to_broadcast((128, QW)),
                op=mybir.AluOpType.add,
            )

        def kproj_chunk(wk_t, kt_t, hp, kn):
            ks = slice(kn * KW, (kn + 1) * KW)
            pj = ppj.tile([128, KW], f32, tag="pj")
            for kc in range(KC):
                nc.tensor.matmul(
                    pj, lhsT=wk_t[:, kc, :], rhs=xk_sb[:, kc, ks],
                    start=(kc == 0), stop=(kc == KC - 1),
                )
            nc.any.tensor_tensor(
                out=kt_t[:, ks], in0=pj,
                in1=bk_sb[:, hp:hp + 1].to_broadcast((128, KW)),
                op=mybir.AluOpType.add,
            )

        def load_wq(hp):
            t = pw.tile([128, KC, 128], bf16, tag="wq")
            nc.sync.dma_start(
                out=t,
                in_=wqTr[hp * D:(hp + 1) * D, :].rearrange(
                    "(c p) m -> p c m", p=128),
            )
            return t

        def load_wk(hp):
            t = pw.tile([128, KC, 128], bf16, tag="wk")
            nc.sync.dma_start(
                out=t,
                in_=wkTr[hp * D:(hp + 1) * D, :].rearrange(
                    "(c p) m -> p c m", p=128),
            )
            return t

        # Software pipeline: projection chunks for head pair hp+1 (and the
        # next group's V) are queued at hp's attention start and drained one
        # per kc iteration, so the exp stream never faces a serial
        # projection-only block at group boundaries.
        work = []

        def push_qk(hp):
            wq_t = load_wq(hp)
            qt_t = pqk.tile([128, SQ], bf16, tag="qt")
            for qn in range(QN):
                work.append(
                    lambda w=wq_t, q=qt_t, h=hp, n=qn: qproj_chunk(w, q, h, n)
                )
            wk_t = load_wk(hp)
            kt_t = pqk.tile([128, SK], bf16, tag="kt")
            for kn in range(KN):
                work.append(
                    lambda w=wk_t, k=kt_t, h=hp, n=kn: kproj_chunk(w, k, h, n)
                )
            return qt_t, kt_t

        def push_v(g):
            wv_t = load_wv(g)
            vg = pv.tile([128, SKT, 256], bf16, tag="vg")
            for t in range(SKT):
                work.append(lambda w=wv_t, v=vg, t_=t: vproj_chunk(w, v, t_))
            return vg

        # prologue: group-0 V projection and head-pair-0 Q/K emitted directly
        vg_cur = pv.tile([128, SKT, 256], bf16, tag="vg")
        for t in range(SKT):
            vproj_chunk(wv_t0, vg_cur, t)
        wq_t = load_wq(0)
        qt_cur = pqk.tile([128, SQ], bf16, tag="qt")
        for qn in range(QN):
            qproj_chunk(wq_t, qt_cur, 0, qn)
        wk_t = load_wk(0)
        kt_cur = pqk.tile([128, SK], bf16, tag="kt")
        for kn in range(KN):
            kproj_chunk(wk_t, kt_cur, 0, kn)

        # out-projection weights: after the hot lead-in transfers
        nc.sync.dma_start(
            out=wo_sb, in_=woTr.rearrange("(h p) e -> p h e", p=128)
        )

        vg_next = None
        for hp in range(HP):
            g, j = divmod(hp, 2)
            nxt = push_qk(hp + 1) if hp + 1 < HP else None
            if j == 0 and g + 1 < NG:
                vg_next = push_v(g + 1)

            # ---- attention for heads A=2*hp, B=2*hp+1
            cx = pcx.tile([128, SQ], f32, tag="cx")
            se = pse.tile([128, QW], f32, tag="se")
            for kc in range(SKT):
                kslice = slice(kc * 128, (kc + 1) * 128)
                sA = psc.tile([128, SQ], f32, tag="sc")
                sB = psc.tile([128, SQ], f32, tag="sc")
                for qn in range(QN):
                    qs = slice(qn * QW, (qn + 1) * QW)
                    nc.tensor.matmul(
                        sA[:, qs], lhsT=kt_cur[0:64, kslice],
                        rhs=qt_cur[0:64, qs], start=True, stop=True,
                    )
                    nc.tensor.matmul(
                        sB[:, qs], lhsT=kt_cur[64:128, kslice],
                        rhs=qt_cur[64:128, qs], start=True, stop=True,
                    )
                eA = pe_.tile([128, SQ], bf16, tag="e")
                eB = pe_.tile([128, SQ], bf16, tag="e")
                nc.scalar.activation(out=eA, in_=sA, func=FT.Exp, scale=0.125)
                nc.scalar.activation(out=eB, in_=sB, func=FT.Exp, scale=0.125)
                st, sp = (kc == 0), (kc == SKT - 1)
                va = vg_cur[:, kc, j * 128:j * 128 + 64]
                vb = vg_cur[:, kc, j * 128 + 64:j * 128 + 128]
                for qn in range(QN):
                    qs = slice(qn * QW, (qn + 1) * QW)
                    # sum-exp strips at partitions (qn,A)->0/32, (qn,B)->64/96
                    nc.tensor.matmul(
                        se[32 * qn:32 * qn + 1, :], lhsT=ones_bf,
                        rhs=eA[:, qs], start=st, stop=sp,
                        tile_position=(0, 32 * qn),
                    )
                    nc.tensor.matmul(
                        se[64 + 32 * qn:64 + 32 * qn + 1, :], lhsT=ones_bf,
                        rhs=eB[:, qs], start=st, stop=sp,
                        tile_position=(0, 64 + 32 * qn),
                    )
                    # context accumulation, heads packed at cols 0-63/64-127
                    nc.tensor.matmul(
                        cx[0:64, qs], lhsT=va, rhs=eA[:, qs],
                        start=st, stop=sp,
                    )
                    nc.tensor.matmul(
                        cx[64:128, qs], lhsT=vb, rhs=eB[:, qs],
                        start=st, stop=sp,
                    )
                if work:
                    work.pop(0)()
            # ---- normalize context by sum-exp, store bf16 into CT
            rc = pmisc.tile([97, QW], f32, tag="rc")
            for qn in range(QN):
                for half in range(2):
                    strip = 64 * half + 32 * qn
                    nc.vector.reciprocal(
                        out=rc[strip:strip + 1, :],
                        in_=se[strip:strip + 1, :],
                    )
            # bounce the reciprocal rows through DRAM to replicate them
            # across partitions (stride-0 partition APs are DMA-only)
            rcd = pdram.tile([97, QW], f32, tag="rcd")
            nc.gpsimd.dma_start(out=rcd, in_=rc)
            # free the ctx PSUM bank immediately (raw copy), normalize
            # in place once the reciprocal broadcast lands
            nc.vector.tensor_copy(out=CT[:, hp, :], in_=cx)
            for qn in range(QN):
                qs = slice(qn * QW, (qn + 1) * QW)
                rcb = pmisc.tile([128, QW], f32, tag="rcb")
                for half in range(2):
                    strip = 64 * half + 32 * qn
                    nc.gpsimd.dma_start(
                        out=rcb[64 * half:64 * half + 64, :],
                        in_=rcd[strip:strip + 1, :].to_broadcast((64, QW)),
                    )
                nc.vector.tensor_mul(
                    out=CT[:, hp, qs], in0=CT[:, hp, qs], in1=rcb
                )
            if nxt is not None:
                qt_cur, kt_cur = nxt
            if j == 1 and vg_next is not None:
                vg_cur = vg_next
                vg_next = None

        # ---- out projection + residual + bias + LayerNorm
        # the attention PSUM pools are dead by now: rotate out-proj psum
        # allocations across them for a 3-deep accumulation pipeline
        _tail_pools = [ppj, psc, pcx]
        for qt in range(SQT):
            rows = slice(qt * 128, (qt + 1) * 128)
            xr = pio.tile([128, D], f32, tag="xr")
            nc.sync.dma_start(out=xr, in_=xres[rows, :])
            x = pio.tile([128, D], f32, tag="x")
            nc.vector.tensor_add(out=x, in0=xr, in1=bo2_sb)
            for en in range(EN):
                es = slice(en * EW, (en + 1) * EW)
                _pool = _tail_pools[(qt * EN + en) % len(_tail_pools)]
                _tag = {id(ppj): "pj", id(psc): "sc", id(pcx): "cx"}[id(_pool)]
                pj = _pool.tile([128, EW], f32, tag=_tag)
                for hp in range(HP):
                    nc.tensor.matmul(
                        pj,
                        lhsT=CT[:, hp, qt * 128:(qt + 1) * 128],
                        rhs=wo_sb[:, hp, es],
                        start=(hp == 0),
                        stop=(hp == HP - 1),
                    )
                nc.vector.tensor_add(out=x[:, es], in0=pj, in1=x[:, es])
            # LayerNorm over the D axis
            stats = pmisc.tile([128, NSUB, 6], f32, tag="bst")
            for sb in range(NSUB):
                ss = slice(sb * (D // NSUB), (sb + 1) * (D // NSUB))
                nc.vector.bn_stats(out=stats[:, sb, :], in_=x[:, ss])
            mv = pmisc.tile([128, 2], f32, tag="mv")
            nc.vector.bn_aggr(out=mv, in_=stats)
            sd = pmisc.tile([128, 2], f32, tag="sd")
            nc.scalar.activation(
                out=sd[:, 0:1], in_=mv[:, 1:2], func=FT.Sqrt, bias=eps_sb
            )
            nc.vector.reciprocal(out=sd[:, 0:1], in_=sd[:, 0:1])
            nc.vector.tensor_mul(out=sd[:, 1:2], in0=mv[:, 0:1], in1=sd[:, 0:1])
            nc.vector.tensor_scalar_mul(out=sd[:, 1:2], in0=sd[:, 1:2], scalar1=-1.0)
            nc.scalar.activation(
                out=x, in_=x, func=FT.Identity, bias=sd[:, 1:2], scale=sd[:, 0:1]
            )
            nc.any.tensor_tensor(out=x, in0=x, in1=gam_sb, op=mybir.AluOpType.mult)
            nc.any.tensor_tensor(out=x, in0=x, in1=bet_sb, op=mybir.AluOpType.add)
            nc.sync.dma_start(out=out_d[rows, :], in_=x)

    nc.compile()
    return nc


def host_inputs(query, key, value, Wq, bq, Wk, bk, Wv, bv, Wo, bo,
                ln_gamma, ln_beta, n_cores):
    """Build per-core input maps (host-side slicing/transpose/bf16 cast)."""
    B, S, D = query.shape
    NH = D // HEAD_DIM
    HP, NG = NH // 2, NH // 4
    SQ = S * B // n_cores
    bf = ml_dtypes.bfloat16

    f = np.asarray
    Wq, Wk, Wv, Wo = f(Wq, np.float32), f(Wk, np.float32), f(Wv, np.float32), f(Wo, np.float32)
    wqTr = np.ascontiguousarray(
        Wq.T.reshape(D, HP, 128).transpose(1, 0, 2)).reshape(HP * D, 128).astype(bf)
    wkTr = np.ascontiguousarray(
        Wk.T.reshape(D, HP, 128).transpose(1, 0, 2)).reshape(HP * D, 128).astype(bf)
    wvTr = np.ascontiguousarray(
        Wv.T.reshape(D, NG, 256).transpose(1, 0, 2)).reshape(NG * D, 256).astype(bf)
    woTr = np.ascontiguousarray(Wo.T).astype(bf)
    bo2 = (np.asarray(bo, np.float32)
           + np.asarray(bv, np.float32) @ Wo.T.astype(np.float32)).reshape(1, D)
    shared = {
        "wqTr": wqTr, "wkTr": wkTr, "wvTr": wvTr, "woTr": woTr,
        "bq": np.asarray(bq, np.float32).reshape(D),
        "bk": np.asarray(bk, np.float32).reshape(D),
        "bo2": np.ascontiguousarray(bo2, np.float32),
        "gam": np.asarray(ln_gamma, np.float32).reshape(1, D),
        "bet": np.asarray(ln_beta, np.float32).reshape(1, D),
    }
    halves = n_cores // B
    in_maps = []
    for c in range(n_cores):
        b, h = c // halves, c % halves
        rows = slice(h * SQ, (h + 1) * SQ)
        m = dict(shared)
        m["xqT"] = np.ascontiguousarray(np.asarray(query[b, rows], np.float32).T).astype(bf)
        m["xres"] = np.ascontiguousarray(np.asarray(query[b, rows], np.float32))
        m["xkT"] = np.ascontiguousarray(np.asarray(key[b], np.float32).T).astype(bf)
        m["xvT"] = np.ascontiguousarray(np.asarray(value[b], np.float32).T).astype(bf)
        in_maps.append(m)
    return in_maps


_BUILT = {}


def _get_built(D, NH, SQ, SK):
    key = (D, NH, SQ, SK)
    if key not in _BUILT:
        _BUILT[key] = build_mha(D, NH, SQ, SK)
    return _BUILT[key]


def make_runner(nc, n_cores=N_CORES):
    """Compile nc into a cached callable: list[in_map] -> list[out_map].

    Mirrors concourse.bass2jax.run_bass_via_pjrt's multi-core path but
    without output-buffer donation, so the jitted function is reusable
    (and re-runnable for timing) across calls.
    """
    import jax
    from jax.sharding import Mesh, PartitionSpec, NamedSharding
    from jax.experimental.shard_map import shard_map
    from concourse import bass2jax

    partition_name = (
        nc.partition_id_tensor.name if nc.partition_id_tensor else None
    )
    in_names, out_names, out_avals = [], [], []
    for alloc in nc.m.functions[0].allocations:
        if not isinstance(alloc, mybir.MemoryLocationSet):
            continue
        name = alloc.memorylocations[0].name
        if alloc.kind == "ExternalInput":
            if name != partition_name:
                in_names.append(name)
        elif alloc.kind == "ExternalOutput":
            out_names.append(name)
            out_avals.append(
                jax.core.ShapedArray(
                    tuple(alloc.tensor_shape), mybir.dt.np(alloc.dtype)
                )
            )
    bass2jax.install_neuronx_cc_hook()
    all_names = in_names + out_names + ([partition_name] if partition_name else [])

    def _body(*args):
        operands = list(args)
        if partition_name is not None:
            operands.append(bass2jax.partition_id_tensor())
        return tuple(
            bass2jax._bass_exec_p.bind(
                *operands,
                out_avals=tuple(out_avals),
                in_names=tuple(all_names),
                out_names=tuple(out_names),
                lowering_input_output_aliases=(),
                sim_require_finite=False,
                sim_require_nnan=False,
                nc=nc,
            )
        )

    devices = jax.devices()[:n_cores]
    mesh = Mesh(np.asarray(devices), ("core",))
    spec = NamedSharding(mesh, PartitionSpec("core"))
    nio = len(in_names) + len(out_names)
    f = jax.jit(
        shard_map(
            _body, mesh=mesh,
            in_specs=(PartitionSpec("core"),) * nio,
            out_specs=(PartitionSpec("core"),) * len(out_names),
            check_rep=False,
        ),
        keep_unused=True,
    )

    def put_inputs(in_maps):
        args = [
            jax.device_put(
                np.concatenate(
                    [np.asarray(in_maps[c][k]) for c in range(n_cores)], 0
                ),
                spec,
            )
            for k in in_names
        ]
        args += [
            jax.device_put(
                np.zeros((n_cores * a.shape[0],) + tuple(a.shape[1:]), a.dtype),
                spec,
            )
            for a in out_avals
        ]
        return args

    def run(in_maps):
        args = put_inputs(in_maps)
        outs = f(*args)
        return [
            {
                name: np.asarray(outs[i]).reshape(
                    n_cores, *out_avals[i].shape
                )[c]
                for i, name in enumerate(out_names)
            }
            for c in range(n_cores)
        ]

    run.jitted = f
    run.put_inputs = put_inputs
    run.out_names = out_names
    return run


_RUNNERS = {}


def _get_runner(D, NH, SQ, SK):
    key = (D, NH, SQ, SK)
    if key not in _RUNNERS:
        _RUNNERS[key] = make_runner(_get_built(D, NH, SQ, SK))
    return _RUNNERS[key]


def kernel(query, key, value, Wq, bq, Wk, bk, Wv, bv, Wo, bo, ln_gamma, ln_beta):
    query = np.asarray(query, np.float32)
    B, S, D = query.shape
    NH = D // HEAD_DIM
    SQ = S * B // N_CORES
    run = _get_runner(D, NH, SQ, S)
    in_maps = host_inputs(query, key, value, Wq, bq, Wk, bk, Wv, bv, Wo, bo,
                          ln_gamma, ln_beta, N_CORES)
    results = run(in_maps)
    out = np.empty((B, S, D), np.float32)
    halves = N_CORES // B
    for c in range(N_CORES):
        b, h = c // halves, c % halves
        out[b, h * SQ:(h + 1) * SQ, :] = results[c]["out"]
    return out


# revision 50
# speedup vs baseline: 1.1966x; 1.1966x over previous
"""Multi-head attention + residual + LayerNorm on 8 Trainium2 NeuronCores.

Problem: B=4, S=2048, D=1024, 16 heads (hd=64), fp32 I/O.

Sharding (no collectives): 8 cores = 4 batches x 2 query-halves.
Core c handles batch b=c//2, query rows h*1024:(h+1)*1024 (h=c%2), producing
the full (1024, 1024) output slice for those tokens. K/V projections for the
whole 2048-token sequence of batch b are computed on both cores of the pair
(the only redundant compute, ~20%).

Per-core kernel layout (all matmul operands bf16, fp32 PSUM accumulate):
  - Q/K projections produce TRANSPOSED outputs QT/KT [d_out-part, token-free]
    so attention scores S^T = K_h @ Q_h^T need no on-chip transposes.
  - V projection produces normal layout V [token-part, d-free].
  - scores^T [k-tok, q-tok] per head -> exp (no max subtraction: |s/8| <~ 2
    for randn inputs) -> E^T bf16.
  - sum_k exp: matmul with a ones[128,1] stationary vector, 4 accumulation
    strips packed at PSUM partitions 0/32/64/96 via tile_position col packing.
  - context^T[d, q] = V_chunk.T-free matmuls accumulated over k chunks, two
    heads packed per PE pass at array columns 0-63 / 64-127.
  - out-projection back to normal layout, + residual + bias, LayerNorm.
  - bv is algebraically folded into the output bias on the host:
    bo2 = bo + bv @ Wo.T (softmax-normalized V bias contributes bv exactly).
"""

import sys

for _p in ("/opt/trn_rl_repo",):
    if _p not in sys.path:
        sys.path.insert(0, _p)

from contextlib import ExitStack

import numpy as np
import ml_dtypes

import concourse.bass as bass
import concourse.mybir as mybir
from concourse import bacc
from concourse.tile import TileContext

EMBED = 1024
HEADS = 16
HEAD_DIM = 64
B_FULL, S_FULL = 4, 2048
N_CORES = 8

f32 = mybir.dt.float32
bf16 = mybir.dt.bfloat16
FT = mybir.ActivationFunctionType


def build_mha(D, NH, SQ, SK, num_devices=N_CORES, dbg=False):
    """Build the per-core Bass module.

    D: embed dim, NH: heads, SQ: query tokens this core owns,
    SK: key/value tokens (full sequence of this core's batch).
    """
    HD = 64
    assert D % 128 == 0 and NH * HD == D
    KC = D // 128          # contraction chunks of 128 input features
    HP = NH // 2           # head-pair chunks (= D//128 output chunks)
    NG = HP // 2           # groups of 2 head pairs (V-proj at 256-col grain)
    SKT = SK // 128        # k-token chunks
    SQT = SQ // 128        # q-token tiles for out-proj/LN
    QN = max(1, SQ // 512)  # 512-wide q tiles
    QW = SQ // QN
    KN = max(1, SK // 512)
    KW = SK // KN
    EN = max(1, D // 512)  # 512-wide out-feature tiles
    EW = D // EN
    NSUB = max(1, D // 512)  # bn_stats subgroups

    nc = bacc.Bacc(
        "TRN2", target_bir_lowering=False, debug=False, num_devices=num_devices
    )

    dp = nc.declare_dram_parameter
    xqT = dp("xqT", [D, SQ], bf16, isOutput=False)
    xres = dp("xres", [SQ, D], f32, isOutput=False)
    xkT = dp("xkT", [D, SK], bf16, isOutput=False)
    xvT = dp("xvT", [D, SK], bf16, isOutput=False)
    wqTr = dp("wqTr", [HP * D, 128], bf16, isOutput=False)   # Wq.T hp-col slices
    wkTr = dp("wkTr", [HP * D, 128], bf16, isOutput=False)
    wvTr = dp("wvTr", [NG * D, 256], bf16, isOutput=False)   # Wv.T group slices
    woTr = dp("woTr", [D, D], bf16, isOutput=False)          # Wo.T
    bq_d = dp("bq", [D], f32, isOutput=False)
    bk_d = dp("bk", [D], f32, isOutput=False)
    bo2_d = dp("bo2", [1, D], f32, isOutput=False)           # bo + bv @ Wo.T
    gam_d = dp("gam", [1, D], f32, isOutput=False)
    bet_d = dp("bet", [1, D], f32, isOutput=False)
    out_d = dp("out", [SQ, D], f32, isOutput=True)
    if dbg:
        dbg_qt = dp("dbg_qt", [D, SQ], bf16, isOutput=True)
        dbg_kt = dp("dbg_kt", [D, SK], bf16, isOutput=True)
        dbg_v = dp("dbg_v", [SK, D], bf16, isOutput=True)
        dbg_ct = dp("dbg_ct", [D, SQ], bf16, isOutput=True)
        dbg_rc = dp("dbg_rc", [NH // 2, 97, SQ // max(1, SQ // 512)], f32, isOutput=True)
        dbg_cx = dp("dbg_cx", [D, SQ], f32, isOutput=True)
        dbg_rcb = dp("dbg_rcb", [NH // 2, 128, SQ], f32, isOutput=True)

    with TileContext(nc) as tc, ExitStack() as ctx:
        consts = ctx.enter_context(tc.tile_pool(name="consts", bufs=1))
        px = ctx.enter_context(tc.tile_pool(name="px", bufs=1))
        pw = ctx.enter_context(tc.tile_pool(name="pw", bufs=1))
        pqk = ctx.enter_context(tc.tile_pool(name="pqk", bufs=2))
        pv = ctx.enter_context(tc.tile_pool(name="pv", bufs=2))
        pct = ctx.enter_context(tc.tile_pool(name="pct", bufs=1))
        pe_ = ctx.enter_context(tc.tile_pool(name="pe", bufs=4))
        pmisc = ctx.enter_context(tc.tile_pool(name="pmisc", bufs=2))
        pio = ctx.enter_context(tc.tile_pool(name="pio", bufs=3))

        pdram = ctx.enter_context(tc.tile_pool(name="pdram", bufs=2, space="DRAM"))
        ppj = ctx.enter_context(tc.tile_pool(name="ppj", bufs=1, space="PSUM"))
        psc = ctx.enter_context(tc.tile_pool(name="psc", bufs=2, space="PSUM"))
        pcx = ctx.enter_context(tc.tile_pool(name="pcx", bufs=1, space="PSUM"))
        pse = ctx.enter_context(tc.tile_pool(name="pse", bufs=1, space="PSUM"))

        # ---- constants
        ones_bf = consts.tile([128, 1], bf16, tag="ones")
        nc.vector.memset(ones_bf, 1.0)
        eps_sb = consts.tile([128, 1], f32, tag="eps")
        nc.vector.memset(eps_sb, 1e-5)
        bq_sb = consts.tile([128, KC], f32, tag="bq")
        nc.sync.dma_start(out=bq_sb, in_=bq_d.rearrange("(c p) -> p c", p=128))
        bk_sb = consts.tile([128, KC], f32, tag="bk")
        nc.sync.dma_start(out=bk_sb, in_=bk_d.rearrange("(c p) -> p c", p=128))

        # ---- stage activations (bf16, pre-transposed on host)
        # xv first: the V projection is the first PE consumer, so its DMA
        # must win the queue race to shorten the kernel lead-in stall
        # chunk-split DMAs: first matmuls start after ~one chunk instead of
        # waiting for the whole monolithic transfer
        # xv split by TOKEN range (not feature chunk): each V-proj tok-tile
        # contracts over all KC feature chunks, so a token-range piece is the
        # unit that unblocks the first matmuls
        xv_sb = px.tile([128, KC, SK], bf16, tag="xv")
        for tr in range(4):
            ts_ = slice(tr * (SK // 4), (tr + 1) * (SK // 4))
            nc.sync.dma_start(
                out=xv_sb[:, :, ts_],
                in_=xvT[:, ts_].rearrange("(c p) m -> p c m", p=128),
            )

        def load_wv(g):
            t = pw.tile([128, KC, 256], bf16, tag="wv")
            nc.sync.dma_start(
                out=t,
                in_=wvTr[g * D:(g + 1) * D, :].rearrange("(c p) m -> p c m", p=128),
            )
            return t

        wv_t0 = load_wv(0)  # before xq/xk so the first PE consumer wins the queues

        xq_sb = px.tile([128, KC, SQ], bf16, tag="xq")
        for kc in range(KC):
            nc.sync.dma_start(
                out=xq_sb[:, kc, :], in_=xqT[kc * 128:(kc + 1) * 128, :]
            )
        xk_sb = px.tile([128, KC, SK], bf16, tag="xk")
        for kc in range(KC):
            nc.sync.dma_start(
                out=xk_sb[:, kc, :], in_=xkT[kc * 128:(kc + 1) * 128, :]
            )

        CT = pct.tile([128, HP, SQ], bf16, tag="ct")
        wo_sb = pct.tile([128, HP, D], bf16, tag="wo")

        # row constants replicated across all 128 partitions at DMA time
        # (stride-0 partition APs are DMA-only). Emitted after the hot input
        # transfers: 1.5MB of replicated writes only needed at the LN tail.
        bo2_sb = consts.tile([128, D], f32, tag="bo2")
        nc.sync.dma_start(out=bo2_sb, in_=bo2_d[:].to_broadcast((128, D)))
        gam_sb = consts.tile([128, D], f32, tag="gam")
        nc.sync.dma_start(out=gam_sb, in_=gam_d[:].to_broadcast((128, D)))
        bet_sb = consts.tile([128, D], f32, tag="bet")
        nc.sync.dma_start(out=bet_sb, in_=bet_d[:].to_broadcast((128, D)))

        def vproj_chunk(wv_t, vg, t):
            pj = ppj.tile([128, 256], f32, tag="pj")
            for kc in range(KC):
                nc.tensor.matmul(
                    pj,
                    lhsT=xv_sb[:, kc, t * 128:(t + 1) * 128],
                    rhs=wv_t[:, kc, :],
                    start=(kc == 0),
                    stop=(kc == KC - 1),
                )
            nc.any.tensor_copy(out=vg[:, t, :], in_=pj)

        def qproj_chunk(wq_t, qt_t, hp, qn):
            qs = slice(qn * QW, (qn + 1) * QW)
            pj = ppj.tile([128, QW], f32, tag="pj")
            for kc in range(KC):
                nc.tensor.matmul(
                    pj, lhsT=wq_t[:, kc, :], rhs=xq_sb[:, kc, qs],
                    start=(kc == 0), stop=(kc == KC - 1),
                )
            nc.any.tensor_tensor(
                out=qt_t[:, qs], in0=pj,
                in1=bq_sb[:, hp:hp + 1].to_broadcast((128, QW)),
                op=mybir.AluOpType.add,
            )

        def kproj_chunk(wk_t, kt_t, hp, kn):
            ks = slice(kn * KW, (kn + 1) * KW)
            pj = ppj.tile([128, KW], f32, tag="pj")
            for kc in range(KC):
                nc.tensor.matmul(
                    pj, lhsT=wk_t[:, kc, :], rhs=xk_sb[:, kc, ks],
                    start=(kc == 0), stop=(kc == KC - 1),
                )
            nc.any.tensor_tensor(
                out=kt_t[:, ks], in0=pj,
                in1=bk_sb[:, hp:hp + 1].to_broadcast((128, KW)),
                op=mybir.AluOpType.add,
            )

        def load_wq(hp):
            t = pw.tile([128, KC, 128], bf16, tag="wq")
            nc.sync.dma_start(
                out=t,
                in_=wqTr[hp * D:(hp + 1) * D, :].rearrange(
                    "(c p) m -> p c m", p=128),
            )
            return t

        def load_wk(hp):
            t = pw.tile([128, KC, 128], bf16, tag="wk")
            nc.sync.dma_start(
                out=t,
                in_=wkTr[hp * D:(hp + 1) * D, :].rearrange(
                    "(c p) m -> p c m", p=128),
            )
            return t

        # Software pipeline: projection chunks for head pair hp+1 (and the
        # next group's V) are queued at hp's attention start and drained one
        # per kc iteration, so the exp stream never faces a serial
        # projection-only block at group boundaries.
        work = []

        def push_qk(hp):
            wq_t = load_wq(hp)
            qt_t = pqk.tile([128, SQ], bf16, tag="qt")
            for qn in range(QN):
                work.append(
                    lambda w=wq_t, q=qt_t, h=hp, n=qn: qproj_chunk(w, q, h, n)
                )
            wk_t = load_wk(hp)
            kt_t = pqk.tile([128, SK], bf16, tag="kt")
            for kn in range(KN):
                work.append(
                    lambda w=wk_t, k=kt_t, h=hp, n=kn: kproj_chunk(w, k, h, n)
                )
            return qt_t, kt_t

        def push_v(g):
            wv_t = load_wv(g)
            vg = pv.tile([128, SKT, 256], bf16, tag="vg")
            for t in range(SKT):
                work.append(lambda w=wv_t, v=vg, t_=t: vproj_chunk(w, v, t_))
            return vg

        # prologue: group-0 V projection and head-pair-0 Q/K emitted directly
        vg_cur = pv.tile([128, SKT, 256], bf16, tag="vg")
        for t in range(SKT):
            vproj_chunk(wv_t0, vg_cur, t)
        wq_t = load_wq(0)
        qt_cur = pqk.tile([128, SQ], bf16, tag="qt")
        for qn in range(QN):
            qproj_chunk(wq_t, qt_cur, 0, qn)
        wk_t = load_wk(0)
        kt_cur = pqk.tile([128, SK], bf16, tag="kt")
        for kn in range(KN):
            kproj_chunk(wk_t, kt_cur, 0, kn)

        # out-projection weights: after the hot lead-in transfers
        nc.sync.dma_start(
            out=wo_sb, in_=woTr.rearrange("(h p) e -> p h e", p=128)
        )

        vg_next = None
        for hp in range(HP):
            g, j = divmod(hp, 2)
            nxt = push_qk(hp + 1) if hp + 1 < HP else None
            if j == 0 and g + 1 < NG:
                vg_next = push_v(g + 1)

            # ---- attention for heads A=2*hp, B=2*hp+1
            cx = pcx.tile([128, SQ], f32, tag="cx")
            se = pse.tile([128, QW], f32, tag="se")
            for kc in range(SKT):
                kslice = slice(kc * 128, (kc + 1) * 128)
                sA = psc.tile([128, SQ], f32, tag="sc")
                sB = psc.tile([128, SQ], f32, tag="sc")
                for qn in range(QN):
                    qs = slice(qn * QW, (qn + 1) * QW)
                    nc.tensor.matmul(
                        sA[:, qs], lhsT=kt_cur[0:64, kslice],
                        rhs=qt_cur[0:64, qs], start=True, stop=True,
                    )
                    nc.tensor.matmul(
                        sB[:, qs], lhsT=kt_cur[64:128, kslice],
                        rhs=qt_cur[64:128, qs], start=True, stop=True,
                    )
                eA = pe_.tile([128, SQ], bf16, tag="e")
                eB = pe_.tile([128, SQ], bf16, tag="e")
                nc.scalar.activation(out=eA, in_=sA, func=FT.Exp, scale=0.125)
                nc.scalar.activation(out=eB, in_=sB, func=FT.Exp, scale=0.125)
                st, sp = (kc == 0), (kc == SKT - 1)
                va = vg_cur[:, kc, j * 128:j * 128 + 64]
                vb = vg_cur[:, kc, j * 128 + 64:j * 128 + 128]
                for qn in range(QN):
                    qs = slice(qn * QW, (qn + 1) * QW)
                    # sum-exp strips at partitions (qn,A)->0/32, (qn,B)->64/96
                    nc.tensor.matmul(
                        se[32 * qn:32 * qn + 1, :], lhsT=ones_bf,
                        rhs=eA[:, qs], start=st, stop=sp,
                        tile_position=(0, 32 * qn),
                    )
                    nc.tensor.matmul(
                        se[64 + 32 * qn:64 + 32 * qn + 1, :], lhsT=ones_bf,
                        rhs=eB[:, qs], start=st, stop=sp,
                        tile_position=(0, 64 + 32 * qn),
                    )
                    # context accumulation, heads packed at cols 0-63/64-127
                    nc.tensor.matmul(
                        cx[0:64, qs], lhsT=va, rhs=eA[:, qs],
                        start=st, stop=sp,
                    )
                    nc.tensor.matmul(
                        cx[64:128, qs], lhsT=vb, rhs=eB[:, qs],
                        start=st, stop=sp,
                    )
                if work:
                    work.pop(0)()
            # ---- normalize context by sum-exp, store bf16 into CT
            # one reciprocal pass over all strips (junk between-strip
            # partitions are never read) instead of 4 serial calls
            rc = pmisc.tile([97, QW], f32, tag="rc")
            nc.vector.reciprocal(out=rc, in_=se[0:97, :])
            # bounce the reciprocal rows through DRAM to replicate them
            # across partitions (stride-0 partition APs are DMA-only)
            rcd = pdram.tile([97, QW], f32, tag="rcd")
            nc.gpsimd.dma_start(out=rcd, in_=rc)
            # free the ctx PSUM bank immediately (raw copy), normalize
            # in place once the reciprocal broadcast lands
            nc.vector.tensor_copy(out=CT[:, hp, :], in_=cx)
            for qn in range(QN):
                qs = slice(qn * QW, (qn + 1) * QW)
                rcb = pmisc.tile([128, QW], f32, tag="rcb")
                for half in range(2):
                    strip = 64 * half + 32 * qn
                    nc.gpsimd.dma_start(
                        out=rcb[64 * half:64 * half + 64, :],
                        in_=rcd[strip:strip + 1, :].to_broadcast((64, QW)),
                    )
                nc.vector.tensor_mul(
                    out=CT[:, hp, qs], in0=CT[:, hp, qs], in1=rcb
                )
            if nxt is not None:
                qt_cur, kt_cur = nxt
            if j == 1 and vg_next is not None:
                vg_cur = vg_next
                vg_next = None

        # ---- out projection + residual + bias + LayerNorm
        # the attention PSUM pools are dead by now: rotate out-proj psum
        # allocations across them for a 3-deep accumulation pipeline
        _tail_pools = [ppj, psc, pcx]
        for qt in range(SQT):
            rows = slice(qt * 128, (qt + 1) * 128)
            xr = pio.tile([128, D], f32, tag="xr")
            nc.sync.dma_start(out=xr, in_=xres[rows, :])
            x = pio.tile([128, D], f32, tag="x")
            nc.vector.tensor_add(out=x, in0=xr, in1=bo2_sb)
            for en in range(EN):
                es = slice(en * EW, (en + 1) * EW)
                _pool = _tail_pools[(qt * EN + en) % len(_tail_pools)]
                _tag = {id(ppj): "pj", id(psc): "sc", id(pcx): "cx"}[id(_pool)]
                pj = _pool.tile([128, EW], f32, tag=_tag)
                for hp in range(HP):
                    nc.tensor.matmul(
                        pj,
                        lhsT=CT[:, hp, qt * 128:(qt + 1) * 128],
                        rhs=wo_sb[:, hp, es],
                        start=(hp == 0),
                        stop=(hp == HP - 1),
                    )
                nc.vector.tensor_add(out=x[:, es], in0=pj, in1=x[:, es])
            # LayerNorm over the D axis
            stats = pmisc.tile([128, NSUB, 6], f32, tag="bst")
            for sb in range(NSUB):
                ss = slice(sb * (D // NSUB), (sb + 1) * (D // NSUB))
                nc.vector.bn_stats(out=stats[:, sb, :], in_=x[:, ss])
            mv = pmisc.tile([128, 2], f32, tag="mv")
            nc.vector.bn_aggr(out=mv, in_=stats)
            sd = pmisc.tile([128, 2], f32, tag="sd")
            nc.scalar.activation(
                out=sd[:, 0:1], in_=mv[:, 1:2], func=FT.Sqrt, bias=eps_sb
            )
            nc.vector.reciprocal(out=sd[:, 0:1], in_=sd[:, 0:1])
            nc.vector.tensor_mul(out=sd[:, 1:2], in0=mv[:, 0:1], in1=sd[:, 0:1])
            nc.vector.tensor_scalar_mul(out=sd[:, 1:2], in0=sd[:, 1:2], scalar1=-1.0)
            nc.scalar.activation(
                out=x, in_=x, func=FT.Identity, bias=sd[:, 1:2], scale=sd[:, 0:1]
            )
            nc.any.tensor_tensor(out=x, in0=x, in1=gam_sb, op=mybir.AluOpType.mult)
            nc.any.tensor_tensor(out=x, in0=x, in1=bet_sb, op=mybir.AluOpType.add)
            nc.sync.dma_start(out=out_d[rows, :], in_=x)

    nc.compile()
    return nc


def host_inputs(query, key, value, Wq, bq, Wk, bk, Wv, bv, Wo, bo,
                ln_gamma, ln_beta, n_cores):
    """Build per-core input maps (host-side slicing/transpose/bf16 cast)."""
    B, S, D = query.shape
    NH = D // HEAD_DIM
    HP, NG = NH // 2, NH // 4
    SQ = S * B // n_cores
    bf = ml_dtypes.bfloat16

    f = np.asarray
    Wq, Wk, Wv, Wo = f(Wq, np.float32), f(Wk, np.float32), f(Wv, np.float32), f(Wo, np.float32)
    wqTr = np.ascontiguousarray(
        Wq.T.reshape(D, HP, 128).transpose(1, 0, 2)).reshape(HP * D, 128).astype(bf)
    wkTr = np.ascontiguousarray(
        Wk.T.reshape(D, HP, 128).transpose(1, 0, 2)).reshape(HP * D, 128).astype(bf)
    wvTr = np.ascontiguousarray(
        Wv.T.reshape(D, NG, 256).transpose(1, 0, 2)).reshape(NG * D, 256).astype(bf)
    woTr = np.ascontiguousarray(Wo.T).astype(bf)
    bo2 = (np.asarray(bo, np.float32)
           + np.asarray(bv, np.float32) @ Wo.T.astype(np.float32)).reshape(1, D)
    shared = {
        "wqTr": wqTr, "wkTr": wkTr, "wvTr": wvTr, "woTr": woTr,
        "bq": np.asarray(bq, np.float32).reshape(D),
        "bk": np.asarray(bk, np.float32).reshape(D),
        "bo2": np.ascontiguousarray(bo2, np.float32),
        "gam": np.asarray(ln_gamma, np.float32).reshape(1, D),
        "bet": np.asarray(ln_beta, np.float32).reshape(1, D),
    }
    halves = n_cores // B
    in_maps = []
    for c in range(n_cores):
        b, h = c // halves, c % halves
        rows = slice(h * SQ, (h + 1) * SQ)
        m = dict(shared)
        m["xqT"] = np.ascontiguousarray(np.asarray(query[b, rows], np.float32).T).astype(bf)
        m["xres"] = np.ascontiguousarray(np.asarray(query[b, rows], np.float32))
        m["xkT"] = np.ascontiguousarray(np.asarray(key[b], np.float32).T).astype(bf)
        m["xvT"] = np.ascontiguousarray(np.asarray(value[b], np.float32).T).astype(bf)
        in_maps.append(m)
    return in_maps


_BUILT = {}


def _get_built(D, NH, SQ, SK):
    key = (D, NH, SQ, SK)
    if key not in _BUILT:
        _BUILT[key] = build_mha(D, NH, SQ, SK)
    return _BUILT[key]


def make_runner(nc, n_cores=N_CORES):
    """Compile nc into a cached callable: list[in_map] -> list[out_map].

    Mirrors concourse.bass2jax.run_bass_via_pjrt's multi-core path but
    without output-buffer donation, so the jitted function is reusable
    (and re-runnable for timing) across calls.
    """
    import jax
    from jax.sharding import Mesh, PartitionSpec, NamedSharding
    from jax.experimental.shard_map import shard_map
    from concourse import bass2jax

    partition_name = (
        nc.partition_id_tensor.name if nc.partition_id_tensor else None
    )
    in_names, out_names, out_avals = [], [], []
    for alloc in nc.m.functions[0].allocations:
        if not isinstance(alloc, mybir.MemoryLocationSet):
            continue
        name = alloc.memorylocations[0].name
        if alloc.kind == "ExternalInput":
            if name != partition_name:
                in_names.append(name)
        elif alloc.kind == "ExternalOutput":
            out_names.append(name)
            out_avals.append(
                jax.core.ShapedArray(
                    tuple(alloc.tensor_shape), mybir.dt.np(alloc.dtype)
                )
            )
    bass2jax.install_neuronx_cc_hook()
    all_names = in_names + out_names + ([partition_name] if partition_name else [])

    def _body(*args):
        operands = list(args)
        if partition_name is not None:
            operands.append(bass2jax.partition_id_tensor())
        return tuple(
            bass2jax._bass_exec_p.bind(
                *operands,
                out_avals=tuple(out_avals),
                in_names=tuple(all_names),
                out_names=tuple(out_names),
                lowering_input_output_aliases=(),
                sim_require_finite=False,
                sim_require_nnan=False,
                nc=nc,
            )
        )

    devices = jax.devices()[:n_cores]
    mesh = Mesh(np.asarray(devices), ("core",))
    spec = NamedSharding(mesh, PartitionSpec("core"))
    nio = len(in_names) + len(out_names)
    f = jax.jit(
        shard_map(
            _body, mesh=mesh,
            in_specs=(PartitionSpec("core"),) * nio,
            out_specs=(PartitionSpec("core"),) * len(out_names),
            check_rep=False,
        ),
        keep_unused=True,
    )

    def put_inputs(in_maps):
        args = [
            jax.device_put(
                np.concatenate(
                    [np.asarray(in_maps[c][k]) for c in range(n_cores)], 0
                ),
                spec,
            )
            for k in in_names
        ]
        args += [
            jax.device_put(
                np.zeros((n_cores * a.shape[0],) + tuple(a.shape[1:]), a.dtype),
                spec,
            )
            for a in out_avals
        ]
        return args

    def run(in_maps):
        args = put_inputs(in_maps)
        outs = f(*args)
        return [
            {
                name: np.asarray(outs[i]).reshape(
                    n_cores, *out_avals[i].shape
                )[c]
                for i, name in enumerate(out_names)
            }
            for c in range(n_cores)
        ]

    run.jitted = f
    run.put_inputs = put_inputs
    run.out_names = out_names
    return run


_RUNNERS = {}


def _get_runner(D, NH, SQ, SK):
    key = (D, NH, SQ, SK)
    if key not in _RUNNERS:
        _RUNNERS[key] = make_runner(_get_built(D, NH, SQ, SK))
    return _RUNNERS[key]


def kernel(query, key, value, Wq, bq, Wk, bk, Wv, bv, Wo, bo, ln_gamma, ln_beta):
    query = np.asarray(query, np.float32)
    B, S, D = query.shape
    NH = D // HEAD_DIM
    SQ = S * B // N_CORES
    run = _get_runner(D, NH, SQ, S)
    in_maps = host_inputs(query, key, value, Wq, bq, Wk, bk, Wv, bv, Wo, bo,
                          ln_gamma, ln_beta, N_CORES)
    results = run(in_maps)
    out = np.empty((B, S, D), np.float32)
    halves = N_CORES // B
    for c in range(N_CORES):
        b, h = c // halves, c % halves
        out[b, h * SQ:(h + 1) * SQ, :] = results[c]["out"]
    return out
